# revision 23
# baseline (speedup 1.0000x reference)
"""Trainium2 Bass kernel for nn_MultiHeadAttention_36009005810143.

Data-parallel over batch B=8 across 8 NeuronCores; projection weights
replicated.  Per core: x [1024,640] -> MHA (10 heads, d=64, strict
causal mask, row 0 = softmax over all keys) -> out [1024,640] * mask.

v4 design notes:
 - input DMAs split across both HWDGE queues (sync: x, scalar: W).
 - only the ub=0 Q/K projections and the V projection run up front;
   the ub=1..4 Q/K projection chunks are deferred into the attention
   head slots to fill PE idle time there (scalar exp is the per-head
   long pole) and keep the HAM clock gate warm at 2.4 GHz.
 - block-causal: only lower-triangle (kb <= qb) 128-blocks of S^T are
   computed / exp'd / used in PV, in S^T [k, q] layout.
 - masking is multiplicative-after-exp and only on the 8 diagonal
   blocks per head: gpsimd affine_select zeroes p[k,q] where q <= k
   (exp(s-10000) == 0 exactly in the fp32 reference, so zeroing is
   exact).  Column q==0 of block (0,0) is preserved: the reference's
   row 0 is softmax(s) over all 1024 keys; its kb>=1 pieces come from
   7 small [128,8] score matmuls (cols 1..7 zeroed) folded into the
   qc=0 PV accumulation.
 - PV keeps V(+ones column) stationary with 512-col moving P chunks
   (high PE duty keeps the HAM clock gate at 2.4 GHz); causal kb
   contributions accumulate into sub-ranges of the [65, 512] psum.
 - per head the [65,1024] result transposes back through the PE into
   one [128, 520] psum tile, drained with a single strided copy into
   qb-major staging; heads are software-pipelined around the S tiles.
"""

import os
import sys
import types

import numpy as np

# The agent image's `antenv` package lacks `axon_hooks`, which
# concourse.bass_utils imports unconditionally when trace=True under
# axon.  Provide it (and register the real NTFF hook when available).
try:
    import antenv

    if not hasattr(antenv, "axon_hooks"):
        _hooks_mod = types.ModuleType("antenv.axon_hooks")
        _hooks_mod._hook = None

        def _set_hook(h):
            _hooks_mod._hook = h

        def _get_hook():
            return _hooks_mod._hook

        _hooks_mod.set_axon_ntff_profile_hook = _set_hook
        _hooks_mod.get_axon_ntff_profile_hook = _get_hook
        sys.modules["antenv.axon_hooks"] = _hooks_mod
        antenv.axon_hooks = _hooks_mod
        try:
            from trn_agent_boot.trn_boot import _ntff_profile_via_ctypes

            _set_hook(_ntff_profile_via_ctypes("/opt/axon/libaxon_pjrt.so"))
        except Exception:
            pass
except Exception:
    pass

import concourse.bass as bass
import concourse.mybir as mybir
import concourse.tile as tile
from concourse import bacc
from concourse.bass_utils import run_bass_kernel_spmd
from concourse.masks import make_identity

F32 = mybir.dt.float32
F16 = mybir.dt.float16
AF = mybir.ActivationFunctionType

B, T, D, U, H, DH = 8, 1024, 640, 640, 10, 64
NTB = T // 128   # 8   q/k/t partition blocks
NDB = D // 128   # 5   contraction blocks for projections
NUB = U // 128   # 5   output-feature blocks
QCW = 512        # q chunk width (moving dim of projection matmuls)
NQC = T // QCW   # 2
VCW = 320        # U chunk width for V projection
NVC = U // VCW   # 2
HPB = 5          # heads per V-chunk (VCW // DH)

# S^T psum tile packing: 5 tiles of [128, 1024] per head, each holding
# (kb, global qstart, ncols) segments.  's0' is the q==0 special block
# (7 kb x 8 cols).  Segments never cross a 512-col psum bank boundary.
S_TILES = [
    [(0, 0, 1024)],
    [(1, 128, 896), (7, 896, 128)],
    [(2, 256, 768), (6, 768, 256)],
    [(3, 384, 640), (5, 640, 384)],
    [(4, 512, 512), ('s0', 0, 56)],
]
# pk (exp'd P, fp16 SBUF) column offsets follow the same packing order
PK_OFF = {}
PK_TILE_OFF = []
PK_COLS = 0
for _tl in S_TILES:
    PK_TILE_OFF.append(PK_COLS)
    for _kb, _qs, _nc in _tl:
        PK_OFF[_kb] = PK_COLS
        PK_COLS += _nc

_CACHE: dict = {}


def _build_module():
    nc = bacc.Bacc("TRN2", target_bir_lowering=False, debug=False, num_devices=B)

    x_d = nc.dram_tensor("x", [T, D], F16, kind="ExternalInput").ap()
    m_d = nc.dram_tensor("mask", [T, 1], F32, kind="ExternalInput").ap()
    wq_d = nc.dram_tensor("Wq", [D, U], F16, kind="ExternalInput").ap()
    wk_d = nc.dram_tensor("Wk", [D, U], F16, kind="ExternalInput").ap()
    wv_d = nc.dram_tensor("Wv", [D, U], F16, kind="ExternalInput").ap()
    out_d = nc.dram_tensor("out", [T, U], F32, kind="ExternalOutput").ap()

    ts = bass.ts

    with tile.TileContext(nc) as tc:
        from contextlib import ExitStack

        with ExitStack() as ctx:
            consts = ctx.enter_context(tc.tile_pool(name="consts", bufs=1))
            sb = ctx.enter_context(tc.tile_pool(name="sb", bufs=1))

            ident = consts.tile([128, 128], F32)
            make_identity(nc, ident[:])
            ident16 = consts.tile([128, 128], F16, tag="ident16", name="ident16")
            nc.vector.tensor_copy(ident16[:], ident[:])

            zeros7 = consts.tile([128, 7], F32, tag="zeros7", name="zeros7")
            nc.vector.memset(zeros7[:], 0.0)

            mask_all = consts.tile([128, NTB], F32, tag="mask", name="mask")
            mask_t = [mask_all[:, tb:tb + 1] for tb in range(NTB)]

            # --- long-lived activations (all fp16 matmul operands) -----
            QT = [sb.tile([128, T], F16, tag=f"QT{i}", name=f"QT{i}") for i in range(NUB)]
            KT = [sb.tile([128, T], F16, tag=f"KT{i}", name=f"KT{i}") for i in range(NUB)]
            # V with a ones-column per head: head h at cols [65h, 65h+64),
            # ones at col 65h+64.
            Vg = [sb.tile([128, H * (DH + 1)], F16, tag=f"Vg{i}", name=f"Vg{i}") for i in range(NTB)]
            # numerator/denominator staging, qb-major: q-block tb at cols
            # [650 tb, 650 (tb+1)), head h at 65h within that (64 nums + den)
            Od = sb.tile([128, NTB * H * (DH + 1)], F32, tag="Od", name="Od")

            # =========== phase 0/1: load (dual queue), project =========
            wx = ctx.enter_context(tc.tile_pool(name="wx", bufs=1))
            # batched input DMAs: few big strided transfers (DMA issue
            # instructions cost ~600ns each on the queue engine)
            Wq_all = wx.tile([128, NDB * U], F16, tag="wq", name="wq")
            Wk_all = wx.tile([128, NDB * U], F16, tag="wk", name="wk")
            Wv_all = wx.tile([128, NDB * U], F16, tag="wv", name="wv")
            Xn_all = wx.tile([128, NTB * D], F16, tag="xn", name="xn")
            Wq = [Wq_all[:, i * U:(i + 1) * U] for i in range(NDB)]
            Wk = [Wk_all[:, i * U:(i + 1) * U] for i in range(NDB)]
            Wv = [Wv_all[:, i * U:(i + 1) * U] for i in range(NDB)]
            Xn = [Xn_all[:, i * D:(i + 1) * D] for i in range(NTB)]
            xT = [wx.tile([128, T], F16, tag=f"xT{i}", name=f"xT{i}") for i in range(NDB)]
            x_r = x_d.rearrange("(t p) d -> p t d", p=128)
            for lo, hi in ((0, 1), (1, 3), (3, 5), (5, 8)):
                nc.sync.dma_start(
                    Xn_all[:, lo * D:hi * D].rearrange("p (t d) -> p t d", d=D),
                    x_r[:, lo:hi, :])
            nc.sync.dma_start(
                mask_all[:].rearrange("p (t c) -> p t c", c=1),
                m_d.rearrange("(t p) c -> p t c", p=128))
            for w_all, w_d in ((Wq_all, wq_d), (Wk_all, wk_d), (Wv_all, wv_d)):
                w_r = w_d.rearrange("(b p) u -> p b u", p=128)
                for lo, hi in ((0, 2), (2, NDB)):
                    nc.scalar.dma_start(
                        w_all[:, lo * U:hi * U].rearrange("p (b u) -> p b u", u=U),
                        w_r[:, lo:hi, :])

            def qk_proj_run(dst, W, ub, qc, pool, tag):
                ps = pool.tile([128, QCW], F32, tag=tag, name=tag)
                for db in range(NDB):
                    nc.tensor.matmul(
                        ps[:],
                        W[db][:, ts(ub, 128)],
                        xT[db][:, ts(qc, QCW)],
                        start=(db == 0), stop=(db == NDB - 1),
                    )
                nc.vector.tensor_copy(dst[ub][:, ts(qc, QCW)], ps[:])

            def v_proj_tb(tb, pool, tag):
                # V natural [T pblock, U chunk], scattered into Vg layout
                for vc in range(NVC):
                    ps = pool.tile([128, QCW], F32, tag=tag, name=tag)
                    ps = ps[:, 0:VCW]
                    for db in range(NDB):
                        nc.tensor.matmul(
                            ps,
                            xT[db][:, ts(tb, 128)],
                            Wv[db][:, ts(vc, VCW)],
                            start=(db == 0), stop=(db == NDB - 1),
                        )
                    dst = Vg[tb][:, vc * HPB * (DH + 1):(vc + 1) * HPB * (DH + 1)]
                    dst = dst.rearrange("p (g c) -> p g c", c=DH + 1)[:, :, 0:DH]
                    src = ps.rearrange("p (g c) -> p g c", c=DH)
                    nc.vector.tensor_copy(dst, src)
                ones_cols = Vg[tb][:].rearrange("p (g c) -> p g c", c=DH + 1)[:, :, DH:DH + 1]
                nc.vector.tensor_copy(
                    ones_cols, ones_t[:].rearrange("p (g c) -> p g c", c=1))

            ones_t = wx.tile([128, H], F32, name="ones_t")
            nc.vector.memset(ones_t[:], 1.0)

            with tc.tile_pool(name="pp", bufs=4, space="PSUM") as pp:
                # dummy matmul stream during the input-DMA wait: keeps the
                # PE busy so the HAM clock gate un-throttles to 2.4 GHz
                # before the real work starts (no data deps, outputs unused)
                warm = wx.tile([128, 512], F16, tag="warm", name="warm")
                nc.vector.memset(warm[:], 0.25)
                for _ in range(10):
                    wp = pp.tile([128, QCW], F32, tag="prj", name="prj")
                    nc.tensor.matmul(
                        wp[:], warm[:, 0:128], warm[:], start=True, stop=True)

                # x^T via PE transpose of 128x128 tiles
                for tb in range(NTB):
                    for db in range(NDB):
                        pt_ = pp.tile([128, 128], F16, tag="trx", name="trx")
                        nc.tensor.matmul(
                            pt_[:], Xn[tb][:, ts(db, 128)], ident16[:],
                            is_transpose=True,
                        )
                        nc.vector.tensor_copy(xT[db][:, ts(tb, 128)], pt_[:])

                # ub=0 Q^T/K^T only (heads 0/1); the rest is deferred into
                # the attention slots
                for dst, W in ((QT, Wq), (KT, Wk)):
                    for qc in range(NQC):
                        qk_proj_run(dst, W, 0, qc, pp, "prj")

                # V projection for the first half of the k blocks; tb 4..7
                # are deferred into attention slots 0/1
                for tb in range(4):
                    v_proj_tb(tb, pp, "prj")

            # ================= phase 2: attention ======================
            # PSUM: sp 2 x [128,1024]f32 (2 banks each) + pv0/pv1
            # [65,512]f32 (1 bank each) + trp [128,528]f16 (1 bank) +
            # prj2 [128,512]f32 (1 bank) = 8 banks.
            with tc.tile_pool(name="pkp", bufs=3) as pkp, \
                 tc.tile_pool(name="otp", bufs=2) as otp, \
                 tc.tile_pool(name="sp", bufs=2, space="PSUM") as sp, \
                 tc.tile_pool(name="pvp", bufs=1, space="PSUM") as pvp, \
                 tc.tile_pool(name="trp", bufs=1, space="PSUM") as trp, \
                 tc.tile_pool(name="pj2", bufs=1, space="PSUM") as pj2:

                def emit_s_tile(kt, qt, pk, ti):
                    """Score matmuls for packed tile ti, exp into pk, and
                    zero the masked parts (diag blocks / s0 junk cols)."""
                    segs = S_TILES[ti]
                    tile_cols = sum(s[2] for s in segs)
                    s_ps = sp.tile([128, 1024], F32, tag="s", name="s")
                    c = 0
                    for kb, qs, ncols in segs:
                        if kb == 's0':
                            for j in range(7):
                                nc.tensor.matmul(
                                    s_ps[:, c + 8 * j: c + 8 * j + 8],
                                    kt[:, ts(j + 1, 128)], qt[:, 0:8],
                                    start=True, stop=True,
                                )
                            c += ncols
                            continue
                        left = ncols
                        q = qs
                        while left > 0:
                            w = min(512 - (c % 512), left, 512)
                            nc.tensor.matmul(
                                s_ps[:, c:c + w], kt[:, ts(kb, 128)],
                                qt[:, q:q + w],
                                start=True, stop=True,
                            )
                            c += w
                            q += w
                            left -= w
                    o = PK_TILE_OFF[ti]
                    nc.scalar.activation(
                        pk[:, o:o + tile_cols], s_ps[:, 0:tile_cols],
                        AF.Exp, scale=0.125)
                    for kb, qs, ncols in segs:
                        if kb == 's0':
                            # zero the junk cols 1..7 of each 8-group
                            dst3 = pk[:, PK_OFF['s0']:PK_OFF['s0'] + 56]
                            dst3 = dst3.rearrange("p (g c) -> p g c", c=8)[:, :, 1:8]
                            nc.vector.tensor_copy(
                                dst3,
                                zeros7[:].rearrange(
                                    "p (g c) -> p g c", g=1
                                ).to_broadcast((128, 7, 7)),
                            )
                            continue
                        # diagonal-block causal mask: zero p[k, q'] where
                        # q' <= k; block (0,0) keeps column 0 (row 0 of the
                        # reference attends all keys)
                        lo = 1 if kb == 0 else 0
                        dsl = pk[:, PK_OFF[kb] + lo:PK_OFF[kb] + 128]
                        nc.gpsimd.affine_select(
                            out=dsl, in_=dsl,
                            compare_op=mybir.AluOpType.is_ge,
                            fill=0.0, base=(0 if kb == 0 else -1),
                            pattern=[[1, 128 - lo]], channel_multiplier=-1,
                        )

                # causal PV sub-range plans per q-chunk:
                # (kb, pk col offset, out col range lo, width)
                PV_PLAN = {
                    0: [(kb, PK_OFF[kb], 128 * kb, 512 - 128 * kb)
                        for kb in range(4)],
                    1: [(kb, PK_OFF[kb] + 512 - 128 * kb, 0, 512)
                        for kb in range(5)] +
                       [(kb, PK_OFF[kb], 128 * kb - 512, 1024 - 128 * kb)
                        for kb in range(5, 8)],
                }

                def emit_pv(pk, vg, qc):
                    """[65, 512] psum: nums+den for q chunk qc, all causal
                    kb accumulated; qc==0 also folds in the q==0 specials."""
                    pv = pvp.tile([65, 512], F32, tag=f"pv{qc}", name="pv")
                    plan = PV_PLAN[qc]
                    for i, (kb, po, ol, w) in enumerate(plan):
                        nc.tensor.matmul(
                            pv[:, ol:ol + w], vg[kb], pk[:, po:po + w],
                            start=(i == 0), stop=(qc == 1 and i == len(plan) - 1),
                        )
                    if qc == 0:
                        for j in range(7):
                            nc.tensor.matmul(
                                pv[:, 0:8], vg[j + 1],
                                pk[:, PK_OFF['s0'] + 8 * j:PK_OFF['s0'] + 8 * j + 8],
                                start=False, stop=(j == 6),
                            )
                    return pv

                def finish_head(h, ot):
                    """transpose [65,1024] ot back to natural layout and
                    stage into qb-major Od with one strided copy."""
                    # 66-col stride keeps each fp16 psum write 4B-aligned
                    tr = trp.tile([128, NTB * (DH + 2)], F16, tag="tr", name="tr")
                    for qb in range(NTB):
                        nc.tensor.matmul(
                            tr[:, qb * (DH + 2):qb * (DH + 2) + DH + 1],
                            ot[:, ts(qb, 128)], ident16[0:DH + 1, 0:DH + 1],
                            is_transpose=True,
                        )
                    dst3 = Od[:].rearrange("p (t c) -> p t c", c=H * (DH + 1))[
                        :, :, 65 * h:65 * h + 65]
                    src3 = tr[:].rearrange("p (t c) -> p t c", c=DH + 2)[:, :, 0:DH + 1]
                    nc.vector.tensor_copy(dst3, src3)

                def drain_pv(ph, pk_prev):
                    pvg = [Vg[kb][:, ph * (DH + 1):(ph + 1) * (DH + 1)]
                           for kb in range(NTB)]
                    ot = otp.tile([DH + 1, T], F16, tag="ot", name="ot")
                    pv0 = emit_pv(pk_prev, pvg, 0)
                    nc.vector.tensor_copy(ot[:, 0:512], pv0[:])
                    yield
                    pv1 = emit_pv(pk_prev, pvg, 1)
                    nc.vector.tensor_copy(ot[:, 512:1024], pv1[:])
                    finish_head(ph, ot)
                    yield

                pks = {}
                for h in range(H):
                    pb, po = h // 2, (h % 2) * DH
                    kt = KT[pb][po:po + DH, :]
                    qt = QT[pb][po:po + DH, :]
                    pk = pkp.tile([128, PK_COLS], F16, tag="pk", name="pk")
                    pks[h] = pk
                    # PV of head h-2 (pipeline depth 2)
                    pv_steps = iter(())
                    if h >= 2:
                        pv_steps = drain_pv(h - 2, pks[h - 2])
                    # deferred Q^T/K^T projection runs: slot h covers
                    # ub = 1 + h//2 (Q on even slots, K on odd slots)
                    prj_runs = []
                    if h < 2 * (NUB - 1):
                        dub = 1 + h // 2
                        dst, W = ((QT, Wq) if h % 2 == 0 else (KT, Wk))
                        prj_runs = [(dst, W, dub, qc) for qc in range(NQC)]
                    emit_s_tile(kt, qt, pk, 0)
                    emit_s_tile(kt, qt, pk, 1)
                    next(pv_steps, None)
                    if prj_runs:
                        qk_proj_run(*prj_runs[0][:2], *prj_runs[0][2:], pj2, "pj2")
                    if h < 2:
                        v_proj_tb(4 + 2 * h, pj2, "pj2")
                    emit_s_tile(kt, qt, pk, 2)
                    emit_s_tile(kt, qt, pk, 3)
                    next(pv_steps, None)
                    if prj_runs:
                        qk_proj_run(*prj_runs[1][:2], *prj_runs[1][2:], pj2, "pj2")
                    if h < 2:
                        v_proj_tb(5 + 2 * h, pj2, "pj2")
                    emit_s_tile(kt, qt, pk, 4)
                for _ in drain_pv(H - 2, pks[H - 2]):
                    pass

                # ====== head 9 + phase 3 (divide, query-mask, store) ====
                # head 9's two PV halves finalize per-qb-group so the DVE /
                # gpsimd divide work and out DMAs overlap the PE's second
                # PV chunk instead of serializing at the very end.
                with tc.tile_pool(name="rcp", bufs=8) as rcp, \
                     tc.tile_pool(name="fop", bufs=4) as fop:

                    def finalize_tb(tb, i):
                        od3 = Od[:, tb * H * (DH + 1):(tb + 1) * H * (DH + 1)]
                        od3 = od3.rearrange("p (h c) -> p h c", c=DH + 1)
                        rc10 = rcp.tile([128, H], F32, tag=f"rc{tb}", name="rc10")
                        nc.vector.reciprocal(
                            rc10[:].rearrange("p (h c) -> p h c", c=1),
                            od3[:, :, DH:DH + 1])
                        nc.vector.tensor_scalar_mul(rc10[:], rc10[:], mask_t[tb])
                        eng = nc.gpsimd if i % 2 == 0 else nc.vector
                        dq = nc.sync if i % 2 == 0 else nc.scalar
                        ot2 = fop.tile([128, U], F32, tag="fo", name="fo")
                        eng.tensor_tensor(
                            ot2[:].rearrange("p (h c) -> p h c", c=DH),
                            od3[:, :, 0:DH],
                            rc10[:].rearrange("p (h c) -> p h c", c=1).to_broadcast(
                                (128, H, DH)),
                            op=mybir.AluOpType.mult,
                        )
                        dq.dma_start(out_d[ts(tb, 128), :], ot2[:])

                    h = H - 1
                    pk9 = pks[h]
                    pvg = [Vg[kb][:, h * (DH + 1):(h + 1) * (DH + 1)]
                           for kb in range(NTB)]
                    ot = otp.tile([DH + 1, T], F16, tag="ot", name="ot")
                    pv0 = emit_pv(pk9, pvg, 0)
                    nc.vector.tensor_copy(ot[:, 0:512], pv0[:])
                    tr = trp.tile([128, NTB * (DH + 2)], F16, tag="tr", name="tr")
                    dst4 = Od[:].rearrange("p (t c) -> p t c", c=H * (DH + 1))
                    src4 = tr[:].rearrange("p (t c) -> p t c", c=DH + 2)
                    for qb in range(4):
                        nc.tensor.matmul(
                            tr[:, qb * (DH + 2):qb * (DH + 2) + DH + 1],
                            ot[:, ts(qb, 128)], ident16[0:DH + 1, 0:DH + 1],
                            is_transpose=True,
                        )
                    nc.vector.tensor_copy(
                        dst4[:, 0:4, 65 * h:65 * h + 65], src4[:, 0:4, 0:DH + 1])
                    pv1 = emit_pv(pk9, pvg, 1)
                    for tb in range(4):
                        finalize_tb(tb, tb)
                    nc.vector.tensor_copy(ot[:, 512:1024], pv1[:])
                    for qb in range(4, NTB):
                        nc.tensor.matmul(
                            tr[:, qb * (DH + 2):qb * (DH + 2) + DH + 1],
                            ot[:, ts(qb, 128)], ident16[0:DH + 1, 0:DH + 1],
                            is_transpose=True,
                        )
                    nc.vector.tensor_copy(
                        dst4[:, 4:8, 65 * h:65 * h + 65], src4[:, 4:8, 0:DH + 1])
                    for tb in range(4, NTB):
                        finalize_tb(tb, tb)

    nc.compile()
    return nc


def get_nc():
    if "nc" not in _CACHE:
        _CACHE["nc"] = _build_module()
    return _CACHE["nc"]


def kernel(x, mask, Wq, Wk, Wv):
    x = np.ascontiguousarray(np.asarray(x, dtype=np.float32).astype(np.float16))
    mask_f = np.ascontiguousarray(
        np.asarray(mask).astype(np.float32).reshape(B, T, 1))
    Wq = np.ascontiguousarray(np.asarray(Wq, dtype=np.float32).astype(np.float16))
    Wk = np.ascontiguousarray(np.asarray(Wk, dtype=np.float32).astype(np.float16))
    Wv = np.ascontiguousarray(np.asarray(Wv, dtype=np.float32).astype(np.float16))

    nc = get_nc()
    in_maps = [
        {"x": x[b], "mask": mask_f[b], "Wq": Wq, "Wk": Wk, "Wv": Wv}
        for b in range(B)
    ]
    trace = bool(int(os.environ.get("KERNEL_TRACE", "0")))
    res = run_bass_kernel_spmd(nc, in_maps, list(range(B)), trace=trace)
    _CACHE["last_results"] = res
    return np.stack([res.results[b]["out"] for b in range(B)], axis=0)
